# revision 1
# baseline (speedup 1.0000x reference)
# Trainium2 Bass kernel for nn_HamEvo_56006373540016.
#
# Math: the reference integrates ds/dt = -i H s with RK4 (10 steps, 4 stages)
# where H acts only on qubits (18, 19) of a 20-qubit state — i.e. a 4x4
# complex matrix per batch element applied along the "s" axis of
# state[x, s, b] (x = 2^18 spectator index, s = 4, b = 16 batch).
# RK4 on a LINEAR ODE is exactly the degree-4 Taylor polynomial of exp(hA),
# so the whole 10-step evolution collapses to one 4x4 complex matrix per
# batch: E_b = (I + hA + (hA)^2/2 + (hA)^3/6 + (hA)^4/24)^10, A = -i G_b.
# We precompute E_b on the host in float64, realify it into an 8x8 real block
# (acting on [re(4); im(4)]), and assemble a 128x128 block-diagonal weight
# over the 16 batches. The device kernel is then a single streamed matmul:
#   Y[128, x] = W[128, 128] @ X[128, x]      (partition dim = (b, c, s))
# which reads the state once and writes it once — memory-bound.
#
# Sharding: the x axis (2^18 values) is split contiguously across 8 cores
# (zero communication; every core gets all batches and the same weight).

import numpy as np

P = 128
B = 16
S = 4
X18 = 1 << 18            # number of x values (qubits 0..17)
NCORES = 8
XC = X18 // NCORES       # 32768 x values per core
FT = 4096                # free elems per DMA tile ([128, FT] f32 = 2 MiB)
MM = 512                 # matmul free dim (one PSUM bank of fp32)

_PERM = np.array([0, 2, 1, 3])  # bit-swap of the 2-qubit index (pyqtorch order)

_NC_CACHE = {}


def _build_nc():
    """Build the Bass program (same SPMD program for all 8 cores)."""
    import concourse.mybir as mybir
    from concourse import bacc
    from concourse.tile import TileContext

    nc = bacc.Bacc(
        "TRN2", target_bir_lowering=False, debug=False, num_devices=NCORES
    )
    w = nc.dram_tensor("w", [P, P], mybir.dt.float32, kind="ExternalInput")
    x = nc.dram_tensor("x", [P, XC], mybir.dt.float32, kind="ExternalInput")
    y = nc.dram_tensor("y", [P, XC], mybir.dt.float32, kind="ExternalOutput")

    PB = 2048  # psum group: 4 banks of 512 fp32, copied in one DVE op
    # Tapered tile sizes: small head tile so compute starts early, small
    # tail tiles so the final load->matmul->copy->store chain is short.
    SIZES = [1024, 1024] + [4096] * 7 + [1024, 512, 512]
    assert sum(SIZES) == XC
    with TileContext(nc) as tc:
        with (
            tc.tile_pool(name="wp", bufs=1) as wp,
            tc.tile_pool(name="xin", bufs=6) as xin,
            tc.tile_pool(name="yout", bufs=5) as yout,
            tc.tile_pool(name="ps", bufs=2, space="PSUM") as ps,
        ):
            # Issue the first state loads before the weight load: the
            # weight isn't needed until the first matmul, and this gets
            # data packets flowing during the pipeline fill.
            xts, base = [], 0
            for ft in SIZES[:2]:
                xt = xin.tile([P, FT], mybir.dt.float32, tag="xt")
                nc.sync.dma_start(xt[:, :ft], x[:, base:base + ft])
                xts.append(xt)
                base += ft
            # Weight load rides the (idle-at-head) second HWDGE ring.
            wt = wp.tile([P, P], mybir.dt.float32)
            nc.scalar.dma_start(wt[:], w[:])
            base = 0
            for fi, ft in enumerate(SIZES):
                if fi < 2:
                    xt = xts[fi]
                else:
                    xt = xin.tile([P, FT], mybir.dt.float32, tag="xt")
                    nc.sync.dma_start(xt[:, :ft], x[:, base:base + ft])
                yt = yout.tile([P, FT], mybir.dt.float32, tag="yt")
                for g in range(0, ft, PB):
                    pb = min(PB, ft - g)
                    pt = ps.tile([P, PB], mybir.dt.float32, tag="pt")
                    for j in range(0, pb, MM):
                        # W is block-diagonal: 4 independent 32x32 blocks
                        # -> 4 concurrent matmuls in distinct array
                        # quadrants via tile_position.
                        for q in range(4):
                            r = slice(32 * q, 32 * (q + 1))
                            nc.tensor.matmul(
                                pt[r, j:j + MM],
                                wt[r, r],
                                xt[r, g + j:g + j + MM],
                                tile_position=(32 * q, 32 * q),
                            )
                    nc.vector.tensor_copy(
                        yt[:, g:g + pb], pt[:, :pb]
                    )
                # Out-DMAs alternate between the two HWDGE rings: mostly
                # the ACT ring (keeps them off the in-DMA ring), but the
                # Sync ring is idle by the tail, so odd late tiles use it
                # to parallelize the final trigger chain.
                out_eng = nc.scalar if (fi < 8 or fi % 2 == 0) else nc.sync
                out_eng.dma_start(y[:, base:base + ft], yt[:, :ft])
                base += ft
    nc.compile()
    return nc


def _get_nc():
    if "nc" not in _NC_CACHE:
        _NC_CACHE["nc"] = _build_nc()
    return _NC_CACHE["nc"]


def _build_weight(H_re, H_im, t):
    """128x128 block-diag weight: per-batch realified 10-step RK4 evolution."""
    H = H_re.astype(np.float64) + 1j * H_im.astype(np.float64)  # (4,4,B)
    G = H[_PERM][:, _PERM]  # memory-order gate: G[s_out, s_in, b]
    # reference computes h = t / 10 in float32
    h = (t.astype(np.float32) / np.float32(10)).astype(np.float64)
    I4 = np.eye(S, dtype=np.complex128)
    W = np.zeros((P, P), np.float64)
    for b in range(B):
        M = (-1j) * h[b] * G[:, :, b]
        R = I4 + M + M @ M / 2 + M @ M @ M / 6 + M @ M @ M @ M / 24
        E = np.linalg.matrix_power(R, 10)
        W[b * 8:(b + 1) * 8, b * 8:(b + 1) * 8] = np.block(
            [[E.real, -E.imag], [E.imag, E.real]]
        )
    return W.astype(np.float32)


LAST_RESULT = None


def _run(inputs, trace=False, trace_cores=None, tmpdir=None):
    global LAST_RESULT
    from concourse.bass_utils import run_bass_kernel_spmd

    W = _build_weight(inputs["H_re"], inputs["H_im"], inputs["t"])
    lhsT = np.ascontiguousarray(W.T)  # matmul computes lhsT.T @ rhs

    # Repack state into [p, x] with p = b*8 + c*4 + s.
    sr = np.asarray(inputs["state_re"], np.float32).reshape(X18, S, B)
    si = np.asarray(inputs["state_im"], np.float32).reshape(X18, S, B)
    A = np.empty((B, 2, S, X18), np.float32)
    A[:, 0] = sr.transpose(2, 1, 0)
    A[:, 1] = si.transpose(2, 1, 0)
    A = A.reshape(P, X18)

    in_maps = [
        {"w": lhsT, "x": np.ascontiguousarray(A[:, c * XC:(c + 1) * XC])}
        for c in range(NCORES)
    ]

    nc = _get_nc()
    res = run_bass_kernel_spmd(
        nc,
        in_maps,
        list(range(NCORES)),
        trace=trace,
        trace_cores=trace_cores,
        tmpdir=tmpdir,
    )
    LAST_RESULT = res

    Y = np.empty((P, X18), np.float32)
    for c in range(NCORES):
        Y[:, c * XC:(c + 1) * XC] = res.results[c]["y"]

    y4 = Y.reshape(B, 2, S, X18)
    out_shape = (2,) * 20 + (B,)
    out = np.empty((2,) + out_shape, np.float32)
    out[0] = y4[:, 0].transpose(2, 1, 0).reshape(out_shape)
    out[1] = y4[:, 1].transpose(2, 1, 0).reshape(out_shape)
    return out, res.exec_time_ns


def kernel(**inputs):
    out, _ = _run(inputs, trace=False)
    return out



# revision 2
# speedup vs baseline: 1.6574x; 1.6574x over previous
# Trainium2 Bass kernel for nn_HamEvo_56006373540016.
#
# Math: the reference integrates ds/dt = -i H s with RK4 (10 steps, 4 stages)
# where H acts only on qubits (18, 19) of a 20-qubit state — i.e. a 4x4
# complex matrix per batch element applied along the "s" axis of
# state[x, s, b] (x = 2^18 spectator index, s = 4, b = 16 batch).
# RK4 on a LINEAR ODE is exactly the degree-4 Taylor polynomial of exp(hA),
# so the whole 10-step evolution collapses to one 4x4 complex matrix per
# batch: E_b = (I + hA + (hA)^2/2 + (hA)^3/6 + (hA)^4/24)^10, A = -i G_b.
# We precompute E_b on the host in float64, realify it into an 8x8 real block
# (acting on [re(4); im(4)]), and assemble a 128x128 block-diagonal weight
# over the 16 batches. The device kernel is then a single streamed matmul:
#   Y[128, x] = W[128, 128] @ X[128, x]      (partition dim = (b, c, s))
# which reads the state once and writes it once — memory-bound.
#
# The whole pipeline runs in bf16 (rel-err budget is 2e-2; bf16 costs ~2e-3):
# X is uploaded as bf16 and Y downloaded as bf16, halving HBM traffic vs f32.
#
# Sharding: the x axis (2^18 values) is split contiguously across 8 cores
# (zero communication; every core gets all batches and the same weight).

import numpy as np
import ml_dtypes

P = 128
B = 16
S = 4
X18 = 1 << 18            # number of x values (qubits 0..17)
NCORES = 8
XC = X18 // NCORES       # 32768 x values per core
FT = 4096                # free elems per DMA tile ([128, FT] bf16 = 1 MiB)
MM = 512                 # matmul free dim (one PSUM bank of fp32)

_PERM = np.array([0, 2, 1, 3])  # bit-swap of the 2-qubit index (pyqtorch order)

_NC_CACHE = {}


def _build_nc():
    """Build the Bass program (same SPMD program for all 8 cores)."""
    import concourse.mybir as mybir
    from concourse import bacc
    from concourse.tile import TileContext

    nc = bacc.Bacc(
        "TRN2", target_bir_lowering=False, debug=False, num_devices=NCORES
    )
    bf16 = mybir.dt.bfloat16
    w = nc.dram_tensor("w", [P, P], bf16, kind="ExternalInput")
    x = nc.dram_tensor("x", [P, XC], bf16, kind="ExternalInput")
    y = nc.dram_tensor("y", [P, XC], bf16, kind="ExternalOutput")

    PB = 2048  # psum group: 4 banks of 512 fp32, copied in one DVE op
    # Tapered tile sizes: small head tile so compute starts early, small
    # tail tiles so the final load->matmul->copy->store chain is short.
    SIZES = [1024, 1024] + [4096] * 7 + [1024, 512, 512]
    assert sum(SIZES) == XC
    with TileContext(nc) as tc:
        with (
            tc.tile_pool(name="wp", bufs=1) as wp,
            tc.tile_pool(name="xin", bufs=6) as xin,
            tc.tile_pool(name="yout", bufs=5) as yout,
            tc.tile_pool(name="ps", bufs=2, space="PSUM") as ps,
        ):
            # Issue the first state loads before the weight load: the
            # weight isn't needed until the first matmul, and this gets
            # data packets flowing during the pipeline fill.
            xts, base = [], 0
            for ft in SIZES[:2]:
                xt = xin.tile([P, FT], bf16, tag="xt")
                nc.sync.dma_start(xt[:, :ft], x[:, base:base + ft])
                xts.append(xt)
                base += ft
            # Weight load rides the (idle-at-head) second HWDGE ring.
            wt = wp.tile([P, P], bf16)
            nc.scalar.dma_start(wt[:], w[:])
            base = 0
            for fi, ft in enumerate(SIZES):
                if fi < 2:
                    xt = xts[fi]
                else:
                    xt = xin.tile([P, FT], bf16, tag="xt")
                    nc.sync.dma_start(xt[:, :ft], x[:, base:base + ft])
                yt = yout.tile([P, FT], bf16, tag="yt")
                for g in range(0, ft, PB):
                    pb = min(PB, ft - g)
                    pt = ps.tile([P, PB], mybir.dt.float32, tag="pt")
                    for j in range(0, pb, MM):
                        # Full-width matmul: W is 128x128 (block-diagonal),
                        # stationary; stream 512-column chunks of X.
                        nc.tensor.matmul(
                            pt[:, j:j + MM],
                            wt[:],
                            xt[:, g + j:g + j + MM],
                        )
                    nc.vector.tensor_copy(
                        yt[:, g:g + pb], pt[:, :pb]
                    )
                # Out-DMAs alternate between the two HWDGE rings: mostly
                # the ACT ring (keeps them off the in-DMA ring), but the
                # Sync ring is idle by the tail, so odd late tiles use it
                # to parallelize the final trigger chain.
                out_eng = nc.scalar if (fi < 8 or fi % 2 == 0) else nc.sync
                out_eng.dma_start(y[:, base:base + ft], yt[:, :ft])
                base += ft
    nc.compile()
    return nc


def _get_nc():
    if "nc" not in _NC_CACHE:
        _NC_CACHE["nc"] = _build_nc()
    return _NC_CACHE["nc"]


def _build_weight(H_re, H_im, t):
    """128x128 block-diag weight: per-batch realified 10-step RK4 evolution."""
    H = H_re.astype(np.float64) + 1j * H_im.astype(np.float64)  # (4,4,B)
    G = H[_PERM][:, _PERM]  # memory-order gate: G[s_out, s_in, b]
    # reference computes h = t / 10 in float32
    h = (t.astype(np.float32) / np.float32(10)).astype(np.float64)
    I4 = np.eye(S, dtype=np.complex128)
    W = np.zeros((P, P), np.float64)
    for b in range(B):
        M = (-1j) * h[b] * G[:, :, b]
        R = I4 + M + M @ M / 2 + M @ M @ M / 6 + M @ M @ M @ M / 24
        E = np.linalg.matrix_power(R, 10)
        W[b * 8:(b + 1) * 8, b * 8:(b + 1) * 8] = np.block(
            [[E.real, -E.imag], [E.imag, E.real]]
        )
    return W.astype(np.float32)


LAST_RESULT = None


def _run(inputs, trace=False, trace_cores=None, tmpdir=None):
    global LAST_RESULT
    from concourse.bass_utils import run_bass_kernel_spmd

    W = _build_weight(inputs["H_re"], inputs["H_im"], inputs["t"])
    lhsT = np.ascontiguousarray(W.T).astype(ml_dtypes.bfloat16)

    # Repack state into [p, x] with p = b*8 + c*4 + s.
    sr = np.asarray(inputs["state_re"], np.float32).reshape(X18, S, B)
    si = np.asarray(inputs["state_im"], np.float32).reshape(X18, S, B)
    A = np.empty((B, 2, S, X18), np.float32)
    A[:, 0] = sr.transpose(2, 1, 0)
    A[:, 1] = si.transpose(2, 1, 0)
    A = A.reshape(P, X18).astype(ml_dtypes.bfloat16)

    in_maps = [
        {"w": lhsT, "x": np.ascontiguousarray(A[:, c * XC:(c + 1) * XC])}
        for c in range(NCORES)
    ]

    nc = _get_nc()
    res = run_bass_kernel_spmd(
        nc,
        in_maps,
        list(range(NCORES)),
        trace=trace,
        trace_cores=trace_cores,
        tmpdir=tmpdir,
    )
    LAST_RESULT = res

    Y = np.empty((P, X18), np.float32)
    for c in range(NCORES):
        Y[:, c * XC:(c + 1) * XC] = np.asarray(
            res.results[c]["y"], dtype=np.float32
        )

    y4 = Y.reshape(B, 2, S, X18)
    out_shape = (2,) * 20 + (B,)
    out = np.empty((2,) + out_shape, np.float32)
    out[0] = y4[:, 0].transpose(2, 1, 0).reshape(out_shape)
    out[1] = y4[:, 1].transpose(2, 1, 0).reshape(out_shape)
    return out, res.exec_time_ns


def kernel(**inputs):
    out, _ = _run(inputs, trace=False)
    return out


# revision 4
# speedup vs baseline: 1.7734x; 1.0700x over previous
# Trainium2 Bass kernel for nn_HamEvo_56006373540016.
#
# Math: the reference integrates ds/dt = -i H s with RK4 (10 steps, 4 stages)
# where H acts only on qubits (18, 19) of a 20-qubit state — i.e. a 4x4
# complex matrix per batch element applied along the "s" axis of
# state[x, s, b] (x = 2^18 spectator index, s = 4, b = 16 batch).
# RK4 on a LINEAR ODE is exactly the degree-4 Taylor polynomial of exp(hA),
# so the whole 10-step evolution collapses to one 4x4 complex matrix per
# batch: E_b = (I + hA + (hA)^2/2 + (hA)^3/6 + (hA)^4/24)^10, A = -i G_b.
# We precompute E_b on the host in float64, realify it into an 8x8 real block
# (acting on [re(4); im(4)]), and assemble a 128x128 block-diagonal weight
# over the 16 batches. The device kernel is then a single streamed matmul:
#   Y[128, x] = W[128, 128] @ X[128, x]      (partition dim = (b, c, s))
# which reads the state once and writes it once — memory-bound.
#
# The whole pipeline runs in bf16 (rel-err budget is 2e-2; bf16 costs ~2e-3):
# X is uploaded as bf16 and Y downloaded as bf16, halving HBM traffic vs f32.
#
# Sharding: the x axis (2^18 values) is split contiguously across 8 cores
# (zero communication; every core gets all batches and the same weight).

import numpy as np
import ml_dtypes

P = 128
B = 16
S = 4
X18 = 1 << 18            # number of x values (qubits 0..17)
NCORES = 8
XC = X18 // NCORES       # 32768 x values per core
FT = 4096                # free elems per DMA tile ([128, FT] bf16 = 1 MiB)
MM = 512                 # matmul free dim (one PSUM bank of fp32)

_PERM = np.array([0, 2, 1, 3])  # bit-swap of the 2-qubit index (pyqtorch order)

_NC_CACHE = {}


def _build_nc():
    """Build the Bass program (same SPMD program for all 8 cores)."""
    import concourse.mybir as mybir
    from concourse import bacc
    from concourse.tile import TileContext

    nc = bacc.Bacc(
        "TRN2", target_bir_lowering=False, debug=False, num_devices=NCORES
    )
    bf16 = mybir.dt.bfloat16
    w = nc.dram_tensor("w", [P, P], bf16, kind="ExternalInput")
    x = nc.dram_tensor("x", [P, XC], bf16, kind="ExternalInput")
    y = nc.dram_tensor("y", [P, XC], bf16, kind="ExternalOutput")

    PB = 2048  # psum group: 4 banks of 512 fp32, copied in one DVE op
    # Tapered tile sizes: small head tile so compute starts early, small
    # tail tiles so the final load->matmul->copy->store chain is short.
    SIZES = [1024, 1024] + [4096] * 7 + [1024, 512, 512]
    assert sum(SIZES) == XC
    with TileContext(nc) as tc:
        with (
            tc.tile_pool(name="wp", bufs=1) as wp,
            tc.tile_pool(name="xin", bufs=6) as xin,
            tc.tile_pool(name="yout", bufs=5) as yout,
            tc.tile_pool(name="ps", bufs=2, space="PSUM") as ps,
        ):
            # Issue the first state loads before the weight load: the
            # weight isn't needed until the first matmul, and this gets
            # data packets flowing during the pipeline fill.
            xts, base = [], 0
            for ft in SIZES[:2]:
                xt = xin.tile([P, FT], bf16, tag="xt")
                nc.sync.dma_start(xt[:, :ft], x[:, base:base + ft])
                xts.append(xt)
                base += ft
            # Weight load rides the (idle-at-head) second HWDGE ring.
            wt = wp.tile([P, P], bf16)
            nc.scalar.dma_start(wt[:], w[:])
            base = 0
            ncopy = 0
            for fi, ft in enumerate(SIZES):
                if fi < 2:
                    xt = xts[fi]
                else:
                    xt = xin.tile([P, FT], bf16, tag="xt")
                    nc.sync.dma_start(xt[:, :ft], x[:, base:base + ft])
                yt = yout.tile([P, FT], bf16, tag="yt")
                for g in range(0, ft, PB):
                    pb = min(PB, ft - g)
                    pt = ps.tile([P, PB], mybir.dt.float32, tag="pt")
                    for j in range(0, pb, MM):
                        # Full-width matmul: W is 128x128 (block-diagonal),
                        # stationary; stream 512-column chunks of X.
                        nc.tensor.matmul(
                            pt[:, j:j + MM],
                            wt[:],
                            xt[:, g + j:g + j + MM],
                        )
                    # The f32->bf16 PSUM evacuation is the serial hot spot
                    # (~2.3us/tile on any one engine); alternate it between
                    # DVE and ACT (GPSIMD cannot read PSUM) so neither
                    # engine becomes the bottleneck.
                    if ncopy % 2 == 0:
                        nc.vector.tensor_copy(yt[:, g:g + pb], pt[:, :pb])
                    else:
                        nc.scalar.copy(yt[:, g:g + pb], pt[:, :pb])
                    ncopy += 1
                # Out-DMAs alternate between the two HWDGE rings: mostly
                # the ACT ring (keeps them off the in-DMA ring), but the
                # Sync ring is idle by the tail, so odd late tiles use it
                # to parallelize the final trigger chain.
                out_eng = nc.scalar if (fi < 8 or fi % 2 == 0) else nc.sync
                out_eng.dma_start(y[:, base:base + ft], yt[:, :ft])
                base += ft
    nc.compile()
    return nc


def _get_nc():
    if "nc" not in _NC_CACHE:
        _NC_CACHE["nc"] = _build_nc()
    return _NC_CACHE["nc"]


def _build_weight(H_re, H_im, t):
    """128x128 block-diag weight: per-batch realified 10-step RK4 evolution."""
    H = H_re.astype(np.float64) + 1j * H_im.astype(np.float64)  # (4,4,B)
    G = H[_PERM][:, _PERM]  # memory-order gate: G[s_out, s_in, b]
    # reference computes h = t / 10 in float32
    h = (t.astype(np.float32) / np.float32(10)).astype(np.float64)
    I4 = np.eye(S, dtype=np.complex128)
    W = np.zeros((P, P), np.float64)
    for b in range(B):
        M = (-1j) * h[b] * G[:, :, b]
        R = I4 + M + M @ M / 2 + M @ M @ M / 6 + M @ M @ M @ M / 24
        E = np.linalg.matrix_power(R, 10)
        W[b * 8:(b + 1) * 8, b * 8:(b + 1) * 8] = np.block(
            [[E.real, -E.imag], [E.imag, E.real]]
        )
    return W.astype(np.float32)


LAST_RESULT = None


def _run(inputs, trace=False, trace_cores=None, tmpdir=None):
    global LAST_RESULT
    from concourse.bass_utils import run_bass_kernel_spmd

    W = _build_weight(inputs["H_re"], inputs["H_im"], inputs["t"])
    lhsT = np.ascontiguousarray(W.T).astype(ml_dtypes.bfloat16)

    # Repack state into [p, x] with p = b*8 + c*4 + s.
    sr = np.asarray(inputs["state_re"], np.float32).reshape(X18, S, B)
    si = np.asarray(inputs["state_im"], np.float32).reshape(X18, S, B)
    A = np.empty((B, 2, S, X18), np.float32)
    A[:, 0] = sr.transpose(2, 1, 0)
    A[:, 1] = si.transpose(2, 1, 0)
    A = A.reshape(P, X18).astype(ml_dtypes.bfloat16)

    in_maps = [
        {"w": lhsT, "x": np.ascontiguousarray(A[:, c * XC:(c + 1) * XC])}
        for c in range(NCORES)
    ]

    nc = _get_nc()
    res = run_bass_kernel_spmd(
        nc,
        in_maps,
        list(range(NCORES)),
        trace=trace,
        trace_cores=trace_cores,
        tmpdir=tmpdir,
    )
    LAST_RESULT = res

    Y = np.empty((P, X18), np.float32)
    for c in range(NCORES):
        Y[:, c * XC:(c + 1) * XC] = np.asarray(
            res.results[c]["y"], dtype=np.float32
        )

    y4 = Y.reshape(B, 2, S, X18)
    out_shape = (2,) * 20 + (B,)
    out = np.empty((2,) + out_shape, np.float32)
    out[0] = y4[:, 0].transpose(2, 1, 0).reshape(out_shape)
    out[1] = y4[:, 1].transpose(2, 1, 0).reshape(out_shape)
    return out, res.exec_time_ns


def kernel(**inputs):
    out, _ = _run(inputs, trace=False)
    return out
